# revision 42
# baseline (speedup 1.0000x reference)
"""Causal attention with L2-normalized Q/K — Trainium2 Bass kernel.

Problem shapes (hardcoded): X [2, 2048, 1024], Wq/Wk/Wv [1024, 1024],
Wo [1024, 1024], bo [1024]; H=16 heads, d_head=64.

Sharding: 8 cores = 2 batches x 4 head-groups (4 heads each).
Core c handles batch b=c//4, heads 4*(c%4)..4*(c%4)+3.
Each core computes QKV projections for its head slice, per-head
normalized causal attention, and a partial output projection
V_hat @ Wo[slice]. The partials are summed with per-q-chunk bf16
ReduceScatters across the 4 cores of the batch; the host reassembles
the row strips.

v2 design notes (vs the f32r baseline at 524us):
- All matmul operands are bf16 (host-converted). On HW, f32r matmuls
  with K=64 contraction ran at ~2x the cycles; bf16 streams 1 cycle/row
  regardless. Also halves DMA bytes and SBUF. Error budget: bf16
  rounding gives ~1e-3 rel vs the 2e-2 gate.
- Q is stored zero-padded per head-half (qt slot h01 has the other
  64 partitions zeroed) so score matmuls contract over the full K=128
  with the natural-layout K tiles.
- No DRAM round-trips for partition broadcasts: rows are broadcast
  across partitions with tiny PE matmuls (sel2 [2,128] selector x
  row [2,512] -> [128,512] in PSUM).
- Q/K norm: sum-of-squares via one ones2-matmul per head pair
  ([2,512]), 1/sqrt via exp(-0.5*ln(x)) on ACT (stays in the
  natural_log_exp table set, no table switching with the attention
  exps). Softmax denominators via DVE reciprocal_approx_fast.
- Causal masking: score/AV matmuls and exps restrict to the valid
  column range; the diagonal 128x128 block gets a triangular mask
  multiply; masked-out pt regions are zero-memset on gpsimd.
- Descending q-chunk order so the biggest attention chunk (and its
  ReduceScatter) runs first; the last RS is the smallest-latency
  exposure. RS runs in bf16 (half the bytes of f32).
- DMAs are batched (one per xt i-tile, one per ypart mc-half) and
  spread across the sync/vector/gpsimd queues; the scalar queue does
  no DMA (ACT is the second-busiest engine).
"""

import math
import numpy as np
from contextlib import ExitStack

import concourse.bass as bass
import concourse.tile as tile
from concourse import mybir
from concourse.bass import _add_dep_helper as add_dep
from concourse.bass_utils import run_bass_kernel_spmd

F32 = mybir.dt.float32
F32R = mybir.dt.float32r
F16 = mybir.dt.float16
AF = mybir.ActivationFunctionType

B, N, D, H, DH = 2, 2048, 1024, 16, 64
NH = 4            # heads per core
J = NH * DH       # head dims per core = 256
P = 128
NQ = 512          # q chunk (moving free dim / psum bank)
NKT = N // P      # 16 k-tiles per head
ID = D // P       # 8 i-tiles of d_model
VW = DH + 1       # 65: V columns + ones column
NQC = N // NQ     # 4 q-chunks

_MAX_WAITS = 1


def _split_excess_waits(nc, limit=_MAX_WAITS):
    """This walrus build allows very few sem waits per instruction.
    Tile can emit many (kernel-tail Drain, collectives reading
    many-writer DRAM). Move excess waits onto injected same-engine
    NoOps right before the instruction; in-order execution preserves
    the semantics."""
    ctr = 0
    for fn in nc.m.functions:
        for bb in fn.blocks:
            out = []
            changed = False
            for ins in bb.instructions:
                si = ins.sync_info
                waits = list(si.on_wait) if si and si.on_wait else []
                if len(waits) > limit:
                    changed = True
                    chunks = [
                        waits[i : i + limit] for i in range(0, len(waits), limit)
                    ]
                    for ch in chunks[:-1]:
                        nop = mybir.InstNoOp(
                            name=f"I-waitsplit-{ctr}", ins=[], outs=[]
                        )
                        ctr += 1
                        nop.engine = ins.engine
                        nop.sync_info = mybir.SyncInfo(on_wait=ch, on_update=[])
                        out.append(nop)
                    ins.sync_info = mybir.SyncInfo(
                        on_wait=chunks[-1], on_update=list(si.on_update or [])
                    )
                out.append(ins)
            if changed:
                bb.instructions = out


def _build():
    nc = bass.Bass("TRN2", target_bir_lowering=False, debug=False, num_devices=8)

    xt = nc.dram_tensor("xt", [ID, NQC, P, NQ], F16, kind="ExternalInput").ap()
    wq = nc.dram_tensor("wq", [2, P, 4, J], F16, kind="ExternalInput").ap()
    wk = nc.dram_tensor("wk", [2, P, 4, J], F16, kind="ExternalInput").ap()
    wv = nc.dram_tensor("wv", [2, P, 4, J], F16, kind="ExternalInput").ap()
    wo = nc.dram_tensor("wo", [P, 2, D], F16, kind="ExternalInput").ap()
    bias4 = nc.dram_tensor("bias4", [D], F32, kind="ExternalInput").ap()
    maskd = nc.dram_tensor("maskd", [P, P], F16, kind="ExternalInput").ap()
    ones2d = nc.dram_tensor("ones2d", [P, 2], F16, kind="ExternalInput").ap()
    sel2d = nc.dram_tensor("sel2d", [2, P], F32R, kind="ExternalInput").ap()
    sel65d = nc.dram_tensor("sel65d", [65, P], F32R, kind="ExternalInput").ap()
    zerosd = nc.dram_tensor("zerosd", [65, NQ], F32R, kind="ExternalInput").ap()
    zb16d = nc.dram_tensor("zb16d", [NQC * NQ], F16, kind="ExternalInput").ap()
    ones1d = nc.dram_tensor("ones1d", [P, 1], F16, kind="ExternalInput").ap()
    # output: per q-chunk, 2 column halves of this core's 128-row strip
    y_ext = nc.dram_tensor(
        "y", [NQC, 2, P, NQ], F16, kind="ExternalOutput"
    ).ap()

    yparts = [
        [nc.dram_tensor(f"ypart{qc}_{mc}", [4, P, NQ], F16) for mc in range(2)]
        for qc in range(NQC)
    ]
    yrss = [
        [nc.dram_tensor(f"yrs{qc}_{mc}", [P, NQ], F16) for mc in range(2)]
        for qc in range(NQC)
    ]

    with tile.TileContext(nc) as tc:
        with ExitStack() as ctx:
            sb = ctx.enter_context(tc.tile_pool(name="sb", bufs=1))
            ps = ctx.enter_context(tc.tile_pool(name="ps", bufs=1, space="PSUM"))

            # ---- loads: batched, spread across sync/vector/gpsimd ----
            ldq = [nc.sync]
            ldi = [0]

            def load(dst_ap, src_ap):
                eng = ldq[ldi[0] % len(ldq)]
                ldi[0] += 1
                eng.dma_start(dst_ap, src_ap)

            def load_w_half(ap_in, nm, h):
                t = sb.tile([P, 4, J], F16, tag=f"{nm}{h}", name=f"{nm}{h}")
                load(t[:], ap_in[h])
                return t

            wq_h = [load_w_half(wq, "wq", h) for h in range(2)]
            wk_h = [load_w_half(wk, "wk", h) for h in range(2)]
            wv_h = [load_w_half(wv, "wv", h) for h in range(2)]
            # xt i-tiles: one DMA each on the gpsimd queue (parallel with
            # the weight loads on sync), all 4 q-chunks [P, NQC, NQ]
            xt_sb = []
            for i in range(ID):
                t = sb.tile([P, NQC, NQ], F16, tag=f"x{i}", name=f"x{i}")
                nc.gpsimd.dma_start(t[:], xt[i].rearrange("c p q -> p c q"))
                xt_sb.append(t)

            wo_sb = sb.tile([P, 2, D], F16, tag="wo")
            load(wo_sb[:], wo)
            bias_sb = sb.tile([P, D], F32, tag="bias")
            load(
                bias_sb[:],
                bias4.rearrange("(a m) -> a m", a=1).to_broadcast((P, D)),
            )
            mask_sb = sb.tile([P, P], F16, tag="mask")
            load(mask_sb[:], maskd)
            ones2_sb = sb.tile([P, 2], F16, tag="ones2")
            load(ones2_sb[:], ones2d)
            sel2_sb = sb.tile([2, P], F32R, tag="sel2")
            load(sel2_sb[:], sel2d)
            sel65_sb = sb.tile([65, P], F32R, tag="sel65")
            load(sel65_sb[:], sel65d)
            # static den staging tile: zeroed once via DMA (memset doesn't
            # codegen for f32r); rows 0/64 are overwritten per use
            den65 = sb.tile([65, NQ], F32R, tag="den65")
            load(den65[:], zerosd)

            # ---- static SBUF state ----
            # qt: zero-padded per h01 slot; kt natural; v with ones col
            qt_sb = sb.tile([P, NQC, 2, NQ], F16, tag="qt")
            kt_sb = sb.tile([P, 2, NQC, NQ], F16, tag="kt")
            v_sb = sb.tile([P, NKT, NH, VW], F16, tag="v")
            # zero the pad slots / set the V ones column via broadcast DMA
            # (memset doesn't run reliably for these shapes)
            zrow3 = zb16d.rearrange("(a c q) -> a c q", a=1, c=NQC)
            load(qt_sb[64:128, :, 0, :], zrow3.to_broadcast((64, NQC, NQ)))
            load(qt_sb[0:64, :, 1, :], zrow3.to_broadcast((64, NQC, NQ)))
            load(
                v_sb[:, :, :, DH : DH + 1].rearrange("p t h x -> p (t h) x"),
                ones1d.rearrange("p (a b) -> p a b", a=1).to_broadcast(
                    (P, NKT * NH, 1)
                ),
            )

            # ---- projections ----
            def proj_qk(w_h, c, is_q):
                for hp in range(2):
                    pp = ps.tile([P, NQ], F32, tag="st", bufs=4)
                    for i in range(ID):
                        nc.tensor.matmul(
                            pp[:],
                            lhsT=w_h[i // 4][:, i % 4, bass.ts(hp, P)],
                            rhs=xt_sb[i][:, c],
                            start=(i == 0),
                            stop=(i == ID - 1),
                        )
                    # fast PSUM evacuation: raw copy + square (both DVE)
                    praw = sb.tile([P, NQ], F16, tag="praw", bufs=3)
                    nc.vector.tensor_copy(praw[:], pp[:])
                    sq = sb.tile([P, NQ], F16, tag="sq", bufs=3)
                    nc.vector.tensor_mul(sq[:], praw[:], praw[:])
                    su = ps.tile([2, NQ], F32, tag="acc", bufs=1)
                    nc.tensor.matmul(
                        su[:], lhsT=ones2_sb[:], rhs=sq[:], start=True, stop=True
                    )
                    # 1/sqrt(x) = exp(-0.5*ln(x)): stays in Exp/Ln table set
                    lnr = sb.tile([2, NQ], F32, tag="lnr", bufs=2)
                    nc.scalar.activation(lnr[:], su[:], AF.Ln)
                    nrm = sb.tile([2, NQ], F32R, tag="nrm", bufs=2)
                    nc.scalar.activation(nrm[:], lnr[:], AF.Exp, scale=-0.5)
                    # partition-broadcast via PE: rows 0-63 <- nrm[0],
                    # rows 64-127 <- nrm[1]
                    rb = ps.tile([P, NQ], F32, tag="acc", bufs=1)
                    nc.tensor.matmul(
                        rb[:], lhsT=sel2_sb[:], rhs=nrm[:], start=True, stop=True
                    )
                    if is_q:
                        nc.vector.tensor_mul(
                            qt_sb[0:64, c, 0, :], praw[0:64, :], rb[0:64, :]
                        )
                        nc.vector.tensor_mul(
                            qt_sb[64:128, c, 1, :], praw[64:128, :], rb[64:128, :]
                        )
                    else:
                        nc.vector.tensor_mul(
                            kt_sb[:, hp, c, :], praw[:], rb[:]
                        )

            v4 = v_sb  # [P, kt, h, VW]

            for c in range(NQC):
                proj_qk(wq_h, c, True)
                proj_qk(wk_h, c, False)
                for tt in range(4):
                    pp = ps.tile([P, J], F32, tag="st", bufs=4)
                    for i in range(ID):
                        nc.tensor.matmul(
                            pp[:],
                            lhsT=xt_sb[i][:, c, bass.ts(tt, P)],
                            rhs=wv_h[i // 4][:, i % 4, :],
                            start=(i == 0),
                            stop=(i == ID - 1),
                        )
                    nc.vector.tensor_copy(
                        v4[:, 4 * c + tt, :, 0:DH],
                        pp[:].rearrange("p (h x) -> p h x", x=DH),
                    )

            # ---- attention + (lagged) output projection + split RS ----
            vhat = {}

            def oproj(qc):
                # per mc half: 4 row-block matmuls, one batched ypart
                # store, then a 0.5MB fp16 ReduceScatter + out DMA
                for mc in range(2):
                    msl = bass.ts(mc, NQ)
                    ysb4 = sb.tile([P, 4, NQ], F16, tag="ysb4", bufs=2)
                    for t4 in range(4):
                        yp = ps.tile([P, NQ], F32, tag="st", bufs=4)
                        for hp in range(2):
                            nc.tensor.matmul(
                                yp[:],
                                lhsT=vhat[(hp, qc)][:, bass.ts(t4, P)],
                                rhs=wo_sb[:, hp, msl],
                                start=(hp == 0),
                                stop=(hp == 1),
                            )
                        nc.vector.tensor_add(
                            ysb4[:, t4, :], yp[:], bias_sb[:, msl]
                        )
                    nc.sync.dma_start(
                        yparts[qc][mc].ap().rearrange("t p q -> p t q"),
                        ysb4[:],
                    )
                    cc = nc.gpsimd.collective_compute(
                        "ReduceScatter",
                        mybir.AluOpType.add,
                        replica_groups=[[0, 1, 2, 3], [4, 5, 6, 7]],
                        ins=[yparts[qc][mc].ap()],
                        outs=[yrss[qc][mc].ap()],
                    )
                    outdma = nc.sync.dma_start(
                        y_ext[qc, mc], yrss[qc][mc].ap()
                    )
                    add_dep(outdma.ins, cc.ins, sync=True, reason="out after rs")

            CH = 2
            for qc in (0, 1, 2, 3):
                nkt = 4 * qc + 4
                for hp in range(2):
                    ots = [
                        ps.tile([P, NQ], F32, tag="ot", bufs=3, name=f"ot{i}")
                        for i in range(2)
                    ]
                    for c0 in range(0, nkt, CH):
                        kts = range(c0, min(c0 + CH, nkt))
                        sts = {}
                        for kt in kts:
                            dj = kt - 4 * qc
                            q0 = P * dj if dj >= 1 else 0
                            for h01 in range(2):
                                st = ps.tile([P, NQ], F32, tag="st", bufs=4)
                                nc.tensor.matmul(
                                    st[:, q0:],
                                    lhsT=kt_sb[:, hp, kt // 4, bass.ts(kt % 4, P)],
                                    rhs=qt_sb[:, qc, h01, q0:],
                                    start=True,
                                    stop=True,
                                )
                                sts[(kt, h01)] = st
                        for kt in kts:
                            dj = kt - 4 * qc
                            q0 = P * dj if dj >= 1 else 0
                            for h01 in range(2):
                                h = 2 * hp + h01
                                # pt[:, 0:q0] is never read (the AV matmul
                                # is range-restricted), so no zeroing
                                pt = sb.tile([P, NQ], F16, tag="pt", bufs=6)
                                nc.scalar.activation(
                                    pt[:, q0:],
                                    sts[(kt, h01)][:, q0:],
                                    AF.Exp,
                                    scale=1.0 / math.sqrt(DH),
                                )
                                if dj >= 0:
                                    blk = slice(P * dj, P * dj + P)
                                    nc.vector.tensor_mul(
                                        pt[:, blk], pt[:, blk], mask_sb[:]
                                    )
                                nc.tensor.matmul(
                                    ots[h01][0:VW, q0:],
                                    lhsT=v_sb[:, kt, h, 0:VW],
                                    rhs=pt[:, q0:],
                                    start=(kt == 0),
                                    stop=(kt == nkt - 1),
                                    skip_group_check=True,
                                )
                    # denominators: evacuate the raw den rows (f32r) to
                    # partitions 0 and 64 of the zeroed [65, NQ] staging
                    # tile (PE operands must start at partition 0/32/64),
                    # then one K=65 selector matmul broadcasts row 0 to
                    # partitions 0-63 and row 64 to partitions 64-127
                    nc.vector.tensor_copy(den65[0:1, :], ots[0][DH : DH + 1, :])
                    nc.vector.tensor_copy(
                        den65[64:65, :], ots[1][DH : DH + 1, :]
                    )
                    rbo = ps.tile([P, NQ], F32, tag="acc", bufs=1)
                    nc.tensor.matmul(
                        rbo[:],
                        lhsT=sel65_sb[:],
                        rhs=den65[:],
                        start=True,
                        stop=True,
                    )
                    # 1/x = exp(-ln(x)) on ACT (same table set as the
                    # attention exps; custom-DVE recip doesn't compile
                    # in this walrus build)
                    lnd = sb.tile([P, NQ], F32, tag="lnd", bufs=2)
                    nc.scalar.activation(lnd[:], rbo[:], AF.Ln)
                    rbos = sb.tile([P, NQ], F32, tag="rbos", bufs=2)
                    nc.scalar.activation(rbos[:], lnd[:], AF.Exp, scale=-1.0)
                    vh = sb.tile(
                        [P, NQ], F16, tag=f"vh{hp}_{qc}", name=f"vh{hp}_{qc}"
                    )
                    nc.vector.tensor_mul(
                        vh[0:64, :], ots[0][0:DH, :], rbos[0:64, :]
                    )
                    nc.vector.tensor_mul(
                        vh[64:128, :], ots[1][0:DH, :], rbos[64:128, :]
                    )
                    vhat[(hp, qc)] = vh

                # output projection lags one chunk behind attention so the
                # den-chain latency is hidden under the next chunk's scores
                if qc > 0:
                    oproj(qc - 1)
            oproj(NQC - 1)

    _split_excess_waits(nc)
    return nc


_NC = None


def _get_nc():
    global _NC
    if _NC is None:
        _NC = _build()
    return _NC


def _make_in_maps(X, Wq, Wk, Wv, Wo, bo):
    BF = np.float16
    X = np.asarray(X, dtype=np.float32)
    Wq = np.asarray(Wq, dtype=np.float32)
    Wk = np.asarray(Wk, dtype=np.float32)
    Wv = np.asarray(Wv, dtype=np.float32)
    Wo = np.asarray(Wo, dtype=np.float32)
    bo = np.asarray(bo, dtype=np.float32)

    r = np.arange(P)
    mask = (r[:, None] <= r[None, :]).astype(BF)
    ones2 = np.zeros((P, 2), dtype=BF)
    ones2[0:64, 0] = 1
    ones2[64:128, 1] = 1
    sel2 = np.zeros((2, P), dtype=np.float32)
    sel2[0, 0:64] = 1
    sel2[1, 64:128] = 1
    sel65 = np.zeros((65, P), dtype=np.float32)
    sel65[0, 0:64] = 1
    sel65[64, 64:128] = 1
    bias4 = (bo * 0.25).astype(np.float32)
    # pre-tiled XT: [i, c, 128, 512] contiguous blocks of X[b].T
    xts = [
        np.ascontiguousarray(
            X[b].T.reshape(ID, P, NQC, NQ).transpose(0, 2, 1, 3)
        ).astype(BF)
        for b in range(B)
    ]

    def wslice(W, jsl):
        # [1024, 256] -> [2, 128, 4, 256] half-major contiguous blocks
        return np.ascontiguousarray(
            W[:, jsl].reshape(2, 4, P, J).transpose(0, 2, 1, 3)
        ).astype(BF)

    in_maps = []
    for c in range(8):
        b, g = c // 4, c % 4
        jsl = slice(g * J, (g + 1) * J)
        in_maps.append(
            {
                "xt": xts[b],
                "wq": wslice(Wq, jsl),
                "wk": wslice(Wk, jsl),
                "wv": wslice(Wv, jsl),
                "wo": np.ascontiguousarray(
                    Wo[jsl, :].reshape(2, P, D).transpose(1, 0, 2)
                ).astype(BF),
                "bias4": bias4,
                "maskd": mask,
                "ones2d": ones2,
                "sel2d": sel2,
                "sel65d": sel65,
                "zerosd": np.zeros((65, NQ), dtype=np.float32),
                "zb16d": np.zeros((NQC * NQ,), dtype=BF),
                "ones1d": np.ones((P, 1), dtype=BF),
            }
        )
    return in_maps


def _gather(res):
    out = np.empty((B, N, D), np.float32)
    for c in range(8):
        b, r = c // 4, c % 4
        yc = np.asarray(res.results[c]["y"], dtype=np.float32)
        for qc in range(NQC):
            rows = slice(NQ * qc + P * r, NQ * qc + P * r + P)
            out[b, rows, 0:NQ] = yc[qc, 0]
            out[b, rows, NQ:D] = yc[qc, 1]
    return out


def kernel(X, Wq, Wk, Wv, Wo, bo):
    nc = _get_nc()
    in_maps = _make_in_maps(X, Wq, Wk, Wv, Wo, bo)
    res = run_bass_kernel_spmd(nc, in_maps, list(range(8)))
    return _gather(res)


# revision 48
# speedup vs baseline: 1.0696x; 1.0696x over previous
"""Causal attention with L2-normalized Q/K — Trainium2 Bass kernel.

Problem shapes (hardcoded): X [2, 2048, 1024], Wq/Wk/Wv [1024, 1024],
Wo [1024, 1024], bo [1024]; H=16 heads, d_head=64.

Sharding: 8 cores = 2 batches x 4 head-groups (4 heads each).
Core c handles batch b=c//4, heads 4*(c%4)..4*(c%4)+3.
Each core computes QKV projections for its head slice, per-head
normalized causal attention, and a partial output projection
V_hat @ Wo[slice]. The partials are summed with per-q-chunk bf16
ReduceScatters across the 4 cores of the batch; the host reassembles
the row strips.

v2 design notes (vs the f32r baseline at 524us):
- All matmul operands are bf16 (host-converted). On HW, f32r matmuls
  with K=64 contraction ran at ~2x the cycles; bf16 streams 1 cycle/row
  regardless. Also halves DMA bytes and SBUF. Error budget: bf16
  rounding gives ~1e-3 rel vs the 2e-2 gate.
- Q is stored zero-padded per head-half (qt slot h01 has the other
  64 partitions zeroed) so score matmuls contract over the full K=128
  with the natural-layout K tiles.
- No DRAM round-trips for partition broadcasts: rows are broadcast
  across partitions with tiny PE matmuls (sel2 [2,128] selector x
  row [2,512] -> [128,512] in PSUM).
- Q/K norm: sum-of-squares via one ones2-matmul per head pair
  ([2,512]), 1/sqrt via exp(-0.5*ln(x)) on ACT (stays in the
  natural_log_exp table set, no table switching with the attention
  exps). Softmax denominators via DVE reciprocal_approx_fast.
- Causal masking: score/AV matmuls and exps restrict to the valid
  column range; the diagonal 128x128 block gets a triangular mask
  multiply; masked-out pt regions are zero-memset on gpsimd.
- Descending q-chunk order so the biggest attention chunk (and its
  ReduceScatter) runs first; the last RS is the smallest-latency
  exposure. RS runs in bf16 (half the bytes of f32).
- DMAs are batched (one per xt i-tile, one per ypart mc-half) and
  spread across the sync/vector/gpsimd queues; the scalar queue does
  no DMA (ACT is the second-busiest engine).
"""

import math
import numpy as np
from contextlib import ExitStack

import concourse.bass as bass
import concourse.tile as tile
from concourse import mybir
from concourse.bass import _add_dep_helper as add_dep
from concourse.bass_utils import run_bass_kernel_spmd

F32 = mybir.dt.float32
F32R = mybir.dt.float32r
F16 = mybir.dt.float16
AF = mybir.ActivationFunctionType

B, N, D, H, DH = 2, 2048, 1024, 16, 64
NH = 4            # heads per core
J = NH * DH       # head dims per core = 256
P = 128
NQ = 512          # q chunk (moving free dim / psum bank)
NKT = N // P      # 16 k-tiles per head
ID = D // P       # 8 i-tiles of d_model
VW = DH + 1       # 65: V columns + ones column
NQC = N // NQ     # 4 q-chunks

_MAX_WAITS = 1


def _split_excess_waits(nc, limit=_MAX_WAITS):
    """This walrus build allows very few sem waits per instruction.
    Tile can emit many (kernel-tail Drain, collectives reading
    many-writer DRAM). Move excess waits onto injected same-engine
    NoOps right before the instruction; in-order execution preserves
    the semantics."""
    ctr = 0
    for fn in nc.m.functions:
        for bb in fn.blocks:
            out = []
            changed = False
            for ins in bb.instructions:
                si = ins.sync_info
                waits = list(si.on_wait) if si and si.on_wait else []
                if len(waits) > limit:
                    changed = True
                    chunks = [
                        waits[i : i + limit] for i in range(0, len(waits), limit)
                    ]
                    for ch in chunks[:-1]:
                        nop = mybir.InstNoOp(
                            name=f"I-waitsplit-{ctr}", ins=[], outs=[]
                        )
                        ctr += 1
                        nop.engine = ins.engine
                        nop.sync_info = mybir.SyncInfo(on_wait=ch, on_update=[])
                        out.append(nop)
                    ins.sync_info = mybir.SyncInfo(
                        on_wait=chunks[-1], on_update=list(si.on_update or [])
                    )
                out.append(ins)
            if changed:
                bb.instructions = out


def _build():
    nc = bass.Bass("TRN2", target_bir_lowering=False, debug=False, num_devices=8)

    xt = nc.dram_tensor("xt", [ID, NQC, P, NQ], F16, kind="ExternalInput").ap()
    wq = nc.dram_tensor("wq", [2, P, 4, J], F16, kind="ExternalInput").ap()
    wk = nc.dram_tensor("wk", [2, P, 4, J], F16, kind="ExternalInput").ap()
    wv = nc.dram_tensor("wv", [2, P, 4, J], F16, kind="ExternalInput").ap()
    wo = nc.dram_tensor("wo", [P, 2, D], F16, kind="ExternalInput").ap()
    bias4 = nc.dram_tensor("bias4", [D], F32, kind="ExternalInput").ap()
    maskd = nc.dram_tensor("maskd", [P, P], F16, kind="ExternalInput").ap()
    ones2d = nc.dram_tensor("ones2d", [P, 2], F16, kind="ExternalInput").ap()
    sel2d = nc.dram_tensor("sel2d", [2, P], F32R, kind="ExternalInput").ap()
    sel65d = nc.dram_tensor("sel65d", [65, P], F32R, kind="ExternalInput").ap()
    zerosd = nc.dram_tensor("zerosd", [65, NQ], F32R, kind="ExternalInput").ap()
    zb16d = nc.dram_tensor("zb16d", [NQC * NQ], F16, kind="ExternalInput").ap()
    ones1d = nc.dram_tensor("ones1d", [P, 1], F16, kind="ExternalInput").ap()
    # output: per q-chunk, 2 column halves of this core's 128-row strip
    y_ext = nc.dram_tensor(
        "y", [NQC, 2, P, NQ], F16, kind="ExternalOutput"
    ).ap()

    yparts = [
        [nc.dram_tensor(f"ypart{qc}_{mc}", [4, P, NQ], F16) for mc in range(2)]
        for qc in range(NQC)
    ]
    yrss = [
        [nc.dram_tensor(f"yrs{qc}_{mc}", [P, NQ], F16) for mc in range(2)]
        for qc in range(NQC)
    ]

    with tile.TileContext(nc) as tc:
        with ExitStack() as ctx:
            sb = ctx.enter_context(tc.tile_pool(name="sb", bufs=1))
            ps = ctx.enter_context(tc.tile_pool(name="ps", bufs=1, space="PSUM"))

            # ---- loads: batched, spread across sync/vector/gpsimd ----
            ldq = [nc.sync]
            ldi = [0]

            def load(dst_ap, src_ap):
                eng = ldq[ldi[0] % len(ldq)]
                ldi[0] += 1
                eng.dma_start(dst_ap, src_ap)

            def load_w_half(ap_in, nm, h):
                t = sb.tile([P, 4, J], F16, tag=f"{nm}{h}", name=f"{nm}{h}")
                load(t[:], ap_in[h])
                return t

            wq_h = [load_w_half(wq, "wq", h) for h in range(2)]
            wk_h = [load_w_half(wk, "wk", h) for h in range(2)]
            wv_h = [load_w_half(wv, "wv", h) for h in range(2)]
            # xt i-tiles: one DMA each, all 4 q-chunks [P, NQC, NQ]
            # (gpsimd SW-DGE loads hang the device — sync queue only)
            xt_sb = []
            for i in range(ID):
                t = sb.tile([P, NQC, NQ], F16, tag=f"x{i}", name=f"x{i}")
                nc.sync.dma_start(t[:], xt[i].rearrange("c p q -> p c q"))
                xt_sb.append(t)

            wo_sb = sb.tile([P, 2, D], F16, tag="wo")
            load(wo_sb[:], wo)
            bias_sb = sb.tile([P, D], F32, tag="bias")
            load(
                bias_sb[:],
                bias4.rearrange("(a m) -> a m", a=1).to_broadcast((P, D)),
            )
            mask_sb = sb.tile([P, P], F16, tag="mask")
            load(mask_sb[:], maskd)
            ones2_sb = sb.tile([P, 2], F16, tag="ones2")
            load(ones2_sb[:], ones2d)
            sel2_sb = sb.tile([2, P], F32R, tag="sel2")
            load(sel2_sb[:], sel2d)
            sel65_sb = sb.tile([65, P], F32R, tag="sel65")
            load(sel65_sb[:], sel65d)
            # static den staging tile: zeroed once via DMA (memset doesn't
            # codegen for f32r); rows 0/64 are overwritten per use
            den65 = sb.tile([65, NQ], F32R, tag="den65")
            load(den65[:], zerosd)

            # ---- static SBUF state (per q-chunk tiles so attention on
            # chunk qc never waits on later chunks' projections — Tile's
            # dependency tracking is tile-granular) ----
            # qt: zero-padded per h01 slot; kt natural; v with ones col
            qt_c = [
                sb.tile([P, 2, NQ], F16, tag=f"qtc{c}", name=f"qtc{c}")
                for c in range(NQC)
            ]
            kt_c = [
                sb.tile([P, 2, NQ], F16, tag=f"ktc{c}", name=f"ktc{c}")
                for c in range(NQC)
            ]
            v_c = [
                sb.tile([P, 4, NH, VW], F16, tag=f"vc{c}", name=f"vc{c}")
                for c in range(NQC)
            ]
            # zero the pad slots / set the V ones column via broadcast DMA
            # (memset doesn't run reliably for these shapes)
            zrow = zb16d.rearrange("(a q) -> a q", a=1)[:, 0:NQ]
            for c in range(NQC):
                load(qt_c[c][64:128, 0, :], zrow.to_broadcast((64, NQ)))
                load(qt_c[c][0:64, 1, :], zrow.to_broadcast((64, NQ)))
                load(
                    v_c[c][:, :, :, DH : DH + 1].rearrange(
                        "p t h x -> p (t h) x"
                    ),
                    ones1d.rearrange("p (a b) -> p a b", a=1).to_broadcast(
                        (P, 4 * NH, 1)
                    ),
                )

            # ---- projections ----
            def proj_qk(w_h, c, is_q):
                for hp in range(2):
                    pp = ps.tile([P, NQ], F32, tag="st", bufs=4)
                    for i in range(ID):
                        nc.tensor.matmul(
                            pp[:],
                            lhsT=w_h[i // 4][:, i % 4, bass.ts(hp, P)],
                            rhs=xt_sb[i][:, c],
                            start=(i == 0),
                            stop=(i == ID - 1),
                        )
                    # fast PSUM evacuation: raw copy + square (both DVE)
                    praw = sb.tile([P, NQ], F16, tag="praw", bufs=3)
                    nc.vector.tensor_copy(praw[:], pp[:])
                    sq = sb.tile([P, NQ], F16, tag="sq", bufs=3)
                    nc.vector.tensor_mul(sq[:], praw[:], praw[:])
                    su = ps.tile([2, NQ], F32, tag="acc", bufs=1)
                    nc.tensor.matmul(
                        su[:], lhsT=ones2_sb[:], rhs=sq[:], start=True, stop=True
                    )
                    # 1/sqrt(x) = exp(-0.5*ln(x)): stays in Exp/Ln table set
                    lnr = sb.tile([2, NQ], F32, tag="lnr", bufs=2)
                    nc.scalar.activation(lnr[:], su[:], AF.Ln)
                    nrm = sb.tile([2, NQ], F32R, tag="nrm", bufs=2)
                    nc.scalar.activation(nrm[:], lnr[:], AF.Exp, scale=-0.5)
                    # partition-broadcast via PE: rows 0-63 <- nrm[0],
                    # rows 64-127 <- nrm[1]
                    rb = ps.tile([P, NQ], F32, tag="acc", bufs=1)
                    nc.tensor.matmul(
                        rb[:], lhsT=sel2_sb[:], rhs=nrm[:], start=True, stop=True
                    )
                    if is_q:
                        nc.vector.tensor_mul(
                            qt_c[c][0:64, 0, :], praw[0:64, :], rb[0:64, :]
                        )
                        nc.vector.tensor_mul(
                            qt_c[c][64:128, 1, :], praw[64:128, :], rb[64:128, :]
                        )
                    else:
                        nc.vector.tensor_mul(
                            kt_c[c][:, hp, :], praw[:], rb[:]
                        )

            def proj(c):
                proj_qk(wq_h, c, True)
                proj_qk(wk_h, c, False)
                for tt in range(4):
                    pp = ps.tile([P, J], F32, tag="st", bufs=4)
                    for i in range(ID):
                        nc.tensor.matmul(
                            pp[:],
                            lhsT=xt_sb[i][:, c, bass.ts(tt, P)],
                            rhs=wv_h[i // 4][:, i % 4, :],
                            start=(i == 0),
                            stop=(i == ID - 1),
                        )
                    nc.vector.tensor_copy(
                        v_c[c][:, tt, :, 0:DH],
                        pp[:].rearrange("p (h x) -> p h x", x=DH),
                    )

            # ---- attention + (lagged) output projection + split RS ----
            vhat = {}

            def oproj(qc):
                # per mc half: 4 row-block matmuls, one batched ypart
                # store, then a 0.5MB fp16 ReduceScatter + out DMA
                for mc in range(2):
                    msl = bass.ts(mc, NQ)
                    ysb4 = sb.tile([P, 4, NQ], F16, tag="ysb4", bufs=2)
                    for t4 in range(4):
                        yp = ps.tile([P, NQ], F32, tag="st", bufs=4)
                        for hp in range(2):
                            nc.tensor.matmul(
                                yp[:],
                                lhsT=vhat[(hp, qc)][:, bass.ts(t4, P)],
                                rhs=wo_sb[:, hp, msl],
                                start=(hp == 0),
                                stop=(hp == 1),
                            )
                        nc.vector.tensor_add(
                            ysb4[:, t4, :], yp[:], bias_sb[:, msl]
                        )
                    nc.sync.dma_start(
                        yparts[qc][mc].ap().rearrange("t p q -> p t q"),
                        ysb4[:],
                    )
                    cc = nc.gpsimd.collective_compute(
                        "ReduceScatter",
                        mybir.AluOpType.add,
                        replica_groups=[[0, 1, 2, 3], [4, 5, 6, 7]],
                        ins=[yparts[qc][mc].ap()],
                        outs=[yrss[qc][mc].ap()],
                    )
                    outdma = nc.sync.dma_start(
                        y_ext[qc, mc], yrss[qc][mc].ap()
                    )
                    add_dep(outdma.ins, cc.ins, sync=True, reason="out after rs")

            CH = 2

            def attn(qc):
                nkt = 4 * qc + 4
                for hp in range(2):
                    ots = [
                        ps.tile([P, NQ], F32, tag="ot", bufs=3, name=f"ot{i}")
                        for i in range(2)
                    ]
                    for c0 in range(0, nkt, CH):
                        kts = range(c0, min(c0 + CH, nkt))
                        sts = {}
                        for kt in kts:
                            dj = kt - 4 * qc
                            q0 = P * dj if dj >= 1 else 0
                            for h01 in range(2):
                                st = ps.tile([P, NQ], F32, tag="st", bufs=4)
                                nc.tensor.matmul(
                                    st[:, q0:],
                                    lhsT=kt_c[kt // 4][:, hp, bass.ts(kt % 4, P)],
                                    rhs=qt_c[qc][:, h01, q0:],
                                    start=True,
                                    stop=True,
                                )
                                sts[(kt, h01)] = st
                        for kt in kts:
                            dj = kt - 4 * qc
                            q0 = P * dj if dj >= 1 else 0
                            for h01 in range(2):
                                h = 2 * hp + h01
                                # pt[:, 0:q0] is never read (the AV matmul
                                # is range-restricted), so no zeroing
                                pt = sb.tile([P, NQ], F16, tag="pt", bufs=6)
                                nc.scalar.activation(
                                    pt[:, q0:],
                                    sts[(kt, h01)][:, q0:],
                                    AF.Exp,
                                    scale=1.0 / math.sqrt(DH),
                                )
                                if dj >= 0:
                                    blk = slice(P * dj, P * dj + P)
                                    nc.vector.tensor_mul(
                                        pt[:, blk], pt[:, blk], mask_sb[:]
                                    )
                                nc.tensor.matmul(
                                    ots[h01][0:VW, q0:],
                                    lhsT=v_c[kt // 4][:, kt % 4, h, 0:VW],
                                    rhs=pt[:, q0:],
                                    start=(kt == 0),
                                    stop=(kt == nkt - 1),
                                    skip_group_check=True,
                                )
                    # denominators: evacuate the raw den rows (f32r) to
                    # partitions 0 and 64 of the zeroed [65, NQ] staging
                    # tile (PE operands must start at partition 0/32/64),
                    # then one K=65 selector matmul broadcasts row 0 to
                    # partitions 0-63 and row 64 to partitions 64-127
                    nc.vector.tensor_copy(den65[0:1, :], ots[0][DH : DH + 1, :])
                    nc.vector.tensor_copy(
                        den65[64:65, :], ots[1][DH : DH + 1, :]
                    )
                    rbo = ps.tile([P, NQ], F32, tag="acc", bufs=1)
                    nc.tensor.matmul(
                        rbo[:],
                        lhsT=sel65_sb[:],
                        rhs=den65[:],
                        start=True,
                        stop=True,
                    )
                    # 1/x = exp(-ln(x)) on ACT (same table set as the
                    # attention exps; custom-DVE recip doesn't compile
                    # in this walrus build)
                    lnd = sb.tile([P, NQ], F32, tag="lnd", bufs=2)
                    nc.scalar.activation(lnd[:], rbo[:], AF.Ln)
                    rbos = sb.tile([P, NQ], F32, tag="rbos", bufs=2)
                    nc.scalar.activation(rbos[:], lnd[:], AF.Exp, scale=-1.0)
                    vh = sb.tile(
                        [P, NQ], F16, tag=f"vh{hp}_{qc}", name=f"vh{hp}_{qc}"
                    )
                    nc.vector.tensor_mul(
                        vh[0:64, :], ots[0][0:DH, :], rbos[0:64, :]
                    )
                    nc.vector.tensor_mul(
                        vh[64:128, :], ots[1][0:DH, :], rbos[64:128, :]
                    )
                    vhat[(hp, qc)] = vh

            # interleaved emission: attention on chunk qc runs while later
            # chunks' projections are still in flight
            proj(0)
            proj(1)
            attn(0)
            oproj(0)
            proj(2)
            attn(1)
            oproj(1)
            proj(3)
            attn(2)
            oproj(2)
            attn(3)
            oproj(3)

    _split_excess_waits(nc)
    return nc


_NC = None


def _get_nc():
    global _NC
    if _NC is None:
        _NC = _build()
    return _NC


def _make_in_maps(X, Wq, Wk, Wv, Wo, bo):
    BF = np.float16
    X = np.asarray(X, dtype=np.float32)
    Wq = np.asarray(Wq, dtype=np.float32)
    Wk = np.asarray(Wk, dtype=np.float32)
    Wv = np.asarray(Wv, dtype=np.float32)
    Wo = np.asarray(Wo, dtype=np.float32)
    bo = np.asarray(bo, dtype=np.float32)

    r = np.arange(P)
    mask = (r[:, None] <= r[None, :]).astype(BF)
    ones2 = np.zeros((P, 2), dtype=BF)
    ones2[0:64, 0] = 1
    ones2[64:128, 1] = 1
    sel2 = np.zeros((2, P), dtype=np.float32)
    sel2[0, 0:64] = 1
    sel2[1, 64:128] = 1
    sel65 = np.zeros((65, P), dtype=np.float32)
    sel65[0, 0:64] = 1
    sel65[64, 64:128] = 1
    bias4 = (bo * 0.25).astype(np.float32)
    # pre-tiled XT: [i, c, 128, 512] contiguous blocks of X[b].T
    xts = [
        np.ascontiguousarray(
            X[b].T.reshape(ID, P, NQC, NQ).transpose(0, 2, 1, 3)
        ).astype(BF)
        for b in range(B)
    ]

    def wslice(W, jsl):
        # [1024, 256] -> [2, 128, 4, 256] half-major contiguous blocks
        return np.ascontiguousarray(
            W[:, jsl].reshape(2, 4, P, J).transpose(0, 2, 1, 3)
        ).astype(BF)

    in_maps = []
    for c in range(8):
        b, g = c // 4, c % 4
        jsl = slice(g * J, (g + 1) * J)
        in_maps.append(
            {
                "xt": xts[b],
                "wq": wslice(Wq, jsl),
                "wk": wslice(Wk, jsl),
                "wv": wslice(Wv, jsl),
                "wo": np.ascontiguousarray(
                    Wo[jsl, :].reshape(2, P, D).transpose(1, 0, 2)
                ).astype(BF),
                "bias4": bias4,
                "maskd": mask,
                "ones2d": ones2,
                "sel2d": sel2,
                "sel65d": sel65,
                "zerosd": np.zeros((65, NQ), dtype=np.float32),
                "zb16d": np.zeros((NQC * NQ,), dtype=BF),
                "ones1d": np.ones((P, 1), dtype=BF),
            }
        )
    return in_maps


def _gather(res):
    out = np.empty((B, N, D), np.float32)
    for c in range(8):
        b, r = c // 4, c % 4
        yc = np.asarray(res.results[c]["y"], dtype=np.float32)
        for qc in range(NQC):
            rows = slice(NQ * qc + P * r, NQ * qc + P * r + P)
            out[b, rows, 0:NQ] = yc[qc, 0]
            out[b, rows, NQ:D] = yc[qc, 1]
    return out


def kernel(X, Wq, Wk, Wv, Wo, bo):
    nc = _get_nc()
    in_maps = _make_in_maps(X, Wq, Wk, Wv, Wo, bo)
    res = run_bass_kernel_spmd(nc, in_maps, list(range(8)))
    return _gather(res)
